# revision 3
# baseline (speedup 1.0000x reference)
"""Trainium2 Bass kernel for nn_Encoder segment-reduce.

Reference computation (per sample b):
    cls = onehot(argmax_k outputs[b])            # [K, HW]
    sizes = cls.sum(HW) + 0.01                   # [K]
    feat_set = feats[b] @ cls.T / sizes          # [F, K]
    out[b] = w_proj @ feat_set + bias            # [E, K]

Kernel strategy (pure data parallel: 1 sample per NeuronCore, 8 cores).

v3: int8 feats, three-engine expansion, fgrp-major stream.

The kernel is jointly limited by the feats HBM stream and the PE pass over
feats (one moving column per f-column per 128-pixel chunk = 65536 PE cycles
~ 27us, the dataflow floor).  bf16 feats make the DMA stream (17MB, ~50us)
the bottleneck; int8 halves it to ~25us, just under the PE floor.  feats
are host-quantized to int8 (scale 127/4.5, ~1e-2 final rel err) and
expanded to bf16 on-chip, with the work split across DVE (~407ns/chunk),
ACT (~712ns/chunk) and GpSimd so the combined cast rate stays ahead of the
PE's 216ns/chunk consumption; 1/s is folded into w_proj on the host.

Loop order is fgrp-major (f-groups of 512 outer, hw chunks inner) so each
f-group's [21, 512] segment-reduce PSUM tile completes after its quarter of
the stream; its PSUM copy, PE transpose back to f-major, and projection
matmuls are interleaved into the FIRST HALF of the next quarter's stream,
keeping every engine's queue stall-free (a queued wait on a future PE
result would stall that engine's later casts).

Tail algebra: the per-class reciprocal commutes with the f-contraction, so
the projection accumulates raw sums into out^T [21, 256]; one recip
multiply plus one bias add (bias host-prebroadcast to [21, 256]) finish in
two DVE ops, and the store is a contiguous 1KB-per-partition DMA (host
transposes).

The onehot is 4 DVE instructions total: tensor_reduce over [P, t, 21] and
a broadcast is_equal via tensor_tensor, in two pieces so the first 8
chunks' onehot is ready early.

DMA: feats ride the sync HWDGE queue as 0.25-1MB sub-blocks (2-8KB
contiguous per partition); outputs/wT/bias/the out store ride the scalar
HWDGE queue so they never delay the feats stream.

A burst of dummy matmuls at kernel start keeps the PE's HAM clock gate warm
through the initial DMA window (cold PE runs at 1.2 GHz vs 2.4 GHz warm;
worse, a multi-us PE idle gap mid-kernel triggers a ~10us half-clock
window, so the PE queue must never go idle once streaming starts).
"""

import numpy as np

import concourse.bacc as bacc
import concourse.bass as bass
import concourse.mybir as mybir
import concourse.tile as tile
from concourse.bass import ds, ts
from concourse.bass_utils import run_bass_kernel_spmd
from concourse.masks import make_identity

# Problem shapes (hardcoded per contract)
B = 8
K = 21
H = 64
W = 64
HW = H * W            # 4096
F = 2048
E = 256
P = 128
FC = F // P           # 16 f-chunks of 128
FG = 4                # f-groups of 512 (psum accumulate tiles)
FGW = F // FG         # 512
N_T = HW // P         # 32 hw chunks
N_CORES = 8

F32 = mybir.dt.float32
BF16 = mybir.dt.bfloat16
I8 = mybir.dt.int8

QCLIP = 4.5
QSCALE = 127.0 / QCLIP

# cast-engine assignment per 16-chunk half: D=DVE, A=ACT, G=GpSimd
CAST_PATTERN = "DGDAGDGDADGDADGA"          # D:7  G:5  A:4


def build_module(warmup=80, cast_pattern=CAST_PATTERN):
    nc = bacc.Bacc("TRN2", target_bir_lowering=False, debug=False)

    # outputs host-transposed to [p, t, k] (pixel-major).
    outputs_d = nc.dram_tensor("outputs_in", [P, N_T, K], F32, kind="ExternalInput")
    # feats int8, fgrp-major: [p, fgrp, t, fj] = int8(featsT[t*128+p, fgrp*512+fj])
    feats_d = nc.dram_tensor("feats_in", [P, FG, N_T, FGW], I8, kind="ExternalInput")
    # (w_proj / s).T rearranged [p, fc, e]
    wT_d = nc.dram_tensor("wT_in", [P, FC, E], BF16, kind="ExternalInput")
    # bias pre-broadcast to [k, e] on host
    bias_d = nc.dram_tensor("bias_in", [K, E], F32, kind="ExternalInput")
    # out^T = [k, e]; host transposes back
    out_d = nc.dram_tensor("out", [K, E], F32, kind="ExternalOutput")

    cast_engines = {"D": "vector", "A": "scalar", "G": "gpsimd"}

    with tile.TileContext(nc) as tc:
        with (
            tc.tile_pool(name="consts", bufs=1) as consts,
            tc.tile_pool(name="fbf", bufs=1) as fbf,
            tc.tile_pool(name="small", bufs=4) as small,
            tc.tile_pool(name="ps_fs", bufs=1, space="PSUM") as ps_fs,
            tc.tile_pool(name="ps_out", bufs=1, space="PSUM") as ps_out,
            tc.tile_pool(name="ps_trp", bufs=1, space="PSUM") as ps_trp,
            tc.tile_pool(name="ps_misc", bufs=1, space="PSUM") as ps_misc,
        ):
            # ---- DMAs ------------------------------------------------
            # scalar HWDGE queue: everything except the feats stream.
            outputs_sb = consts.tile([P, N_T, K], F32)
            nc.scalar.dma_start(out=outputs_sb, in_=outputs_d.ap())
            bias_sb = consts.tile([K, E], F32)
            nc.scalar.dma_start(out=bias_sb, in_=bias_d.ap())
            wT_sb = consts.tile([P, FC, E], BF16)
            nc.scalar.dma_start(out=wT_sb, in_=wT_d.ap())

            # sync HWDGE queue: the feats stream.  fgrp 0 in finer
            # sub-blocks so the first casts can start sooner.
            feats_sb = consts.tile([P, FG, N_T, FGW], I8)
            feats_r = feats_d.ap()
            sub_blocks = {0: [(0, 4), (4, 8), (8, 16), (16, 24), (24, 32)],
                          1: [(0, 16), (16, 32)],
                          2: [(0, 16), (16, 32)],
                          3: [(0, 16), (16, 32)]}
            for g in range(FG):
                for (t0, t1) in sub_blocks[g]:
                    nc.sync.dma_start(
                        out=feats_sb[:, g, ds(t0, t1 - t0)],
                        in_=feats_r[:, g, ds(t0, t1 - t0)],
                    )

            # ---- PE warm-up + constants ------------------------------
            warm_w = consts.tile([P, 64], BF16)
            nc.vector.memset(warm_w, 0.0)
            warm_ps = ps_misc.tile([P, 64], F32, tag="warm")
            for _ in range(warmup):
                nc.tensor.matmul(warm_ps[0:64, :], lhsT=warm_w, rhs=warm_w)

            # Preload the ACT engine's Copy activation table so the first
            # real cast doesn't eat the ~1.3us table load mid-stream.
            act_warm = small.tile([1, 1], BF16, tag="actw")
            nc.scalar.activation(out=act_warm, in_=warm_w[0:1, 0:1],
                                 func=mybir.ActivationFunctionType.Copy)

            ident = consts.tile([P, P], F32)
            make_identity(nc, ident)
            ident_b = consts.tile([K, K], BF16)
            nc.vector.tensor_copy(ident_b, ident[:K, :K])
            ones_b = consts.tile([P, 2], BF16)
            nc.vector.memset(ones_b, 1.0)

            # ---- onehot (DVE, 4 instructions in 2 pieces) ------------
            oh_all = consts.tile([P, N_T, K], BF16)
            rowmax = consts.tile([P, N_T, 1], F32)

            def emit_onehot(t0, t1):
                n = t1 - t0
                nc.vector.tensor_reduce(
                    rowmax[:, ds(t0, n)], outputs_sb[:, ds(t0, n)],
                    mybir.AxisListType.X, mybir.AluOpType.max,
                )
                nc.vector.tensor_tensor(
                    oh_all[:, ds(t0, n)], outputs_sb[:, ds(t0, n)],
                    rowmax[:, ds(t0, n)].to_broadcast((P, n, K)),
                    mybir.AluOpType.is_equal,
                )

            # ---- stream tiles ----------------------------------------
            fg_bf = [
                fbf.tile([P, N_T, FGW], BF16, name=f"fgbf{i}", tag=f"fgbf{i}")
                for i in range(2)
            ]
            fs_ps = [
                ps_fs.tile([K, FGW], F32, name=f"fs{i}", tag=f"fs{i}")
                for i in range(2)
            ]
            fs_sc = consts.tile([K, F], BF16)
            fsT_sb = consts.tile([P, FC, K], BF16)
            sz_ps = ps_misc.tile([K, 2], F32, tag="sz")
            outT_ps = ps_out.tile([K, E], F32)

            def emit_cast(g, t):
                eng = cast_engines[cast_pattern[t % 16]]
                bf = fg_bf[g % 2]
                if eng == "vector":
                    nc.vector.tensor_copy(bf[:, t, :], feats_sb[:, g, t, :])
                elif eng == "gpsimd":
                    nc.gpsimd.tensor_copy(bf[:, t, :], feats_sb[:, g, t, :])
                else:
                    nc.scalar.activation(
                        out=bf[:, t, :], in_=feats_sb[:, g, t, :],
                        func=mybir.ActivationFunctionType.Copy,
                    )

            def emit_stream(g, t0, t1):
                bf = fg_bf[g % 2]
                for t in range(t0, t1):
                    nc.tensor.matmul(
                        fs_ps[g % 2], lhsT=oh_all[:, t, :], rhs=bf[:, t, :],
                        start=(t == 0), stop=(t == N_T - 1),
                    )

            def emit_fs_copy(g):
                nc.vector.tensor_copy(fs_sc[:, ds(g * FGW, FGW)], fs_ps[g % 2])

            def emit_transposes(g):
                for j in range(4):
                    fc = g * 4 + j
                    trp = ps_trp.tile([P, K], BF16, name=f"trp{fc}",
                                      tag=f"trp{'AB'[fc % 2]}")
                    nc.tensor.transpose(trp, fs_sc[:, ts(fc, P)], ident_b)
                    nc.vector.tensor_copy(fsT_sb[:, fc, :], trp)

            def emit_projs(g):
                for j in range(4):
                    fc = g * 4 + j
                    nc.tensor.matmul(
                        outT_ps, lhsT=fsT_sb[:, fc, :], rhs=wT_sb[:, fc, :],
                        start=(fc == 0), stop=(fc == FC - 1),
                    )

            # ---- main schedule ---------------------------------------
            # fgrp 0: onehot piece A, early casts, stream starts; the 32
            # sizes matmuls fill the PE while casts get ahead.
            emit_onehot(0, 8)
            for t in range(8):
                emit_cast(0, t)
            emit_onehot(8, N_T)
            emit_stream(0, 0, 8)
            for t in range(N_T):
                nc.tensor.matmul(
                    sz_ps, lhsT=oh_all[:, t, :], rhs=ones_b,
                    start=(t == 0), stop=(t == N_T - 1),
                )
            for t in range(8, N_T):
                emit_cast(0, t)
            emit_stream(0, 8, N_T)

            # fgrps 1..3: previous fgrp's copy/transpose/proj interleave
            # into the first half of this fgrp's stream.
            for g in range(1, FG):
                for t in range(8):
                    emit_cast(g, t)
                emit_fs_copy(g - 1)
                emit_stream(g, 0, 8)
                emit_transposes(g - 1)
                for t in range(8, 16):
                    emit_cast(g, t)
                emit_stream(g, 8, 16)
                emit_projs(g - 1)
                for t in range(16, N_T):
                    emit_cast(g, t)
                emit_stream(g, 16, N_T)

            emit_fs_copy(FG - 1)
            emit_transposes(FG - 1)
            emit_projs(FG - 1)

            # ---- tail ------------------------------------------------
            sizes_sb = small.tile([K, 1], F32, tag="sizes")
            nc.vector.tensor_scalar_add(sizes_sb, sz_ps[:, 0:1], 0.01)
            recip = small.tile([K, 1], F32, tag="recip")
            nc.vector.reciprocal(recip, sizes_sb)
            out_sb = consts.tile([K, E], F32)
            nc.vector.tensor_scalar_mul(out_sb, outT_ps, recip)
            nc.vector.tensor_add(out_sb, out_sb, bias_sb)
            nc.scalar.dma_start(out=out_d.ap(), in_=out_sb)

    nc.compile()
    return nc


_CACHE = {}


def make_in_maps(outputs, feats, w_proj, b_proj):
    import ml_dtypes

    outputs = np.asarray(outputs, dtype=np.float32)
    # [B, K, H, W] -> per sample [p, t, k] (pixel-major: hw = t*128 + p)
    outputs_t = np.ascontiguousarray(
        outputs.reshape(B, K, N_T, P).transpose(0, 3, 2, 1)
    )
    feats = np.asarray(feats, dtype=np.float32)
    q = np.clip(np.round(feats * QSCALE), -127, 127).astype(np.int8)
    # [B, F, H, W] -> per sample [p, fgrp, t, fj] = q[t*128+p, fgrp*512+fj]
    feats_sh = np.ascontiguousarray(
        q.reshape(B, FG, FGW, N_T, P).transpose(0, 4, 1, 3, 2)
    )
    wT = np.ascontiguousarray(
        (np.asarray(w_proj, dtype=np.float32).T / QSCALE)
        .reshape(FC, P, E).transpose(1, 0, 2)
        .astype(ml_dtypes.bfloat16)
    )
    bias = np.ascontiguousarray(
        np.broadcast_to(np.asarray(b_proj, dtype=np.float32)[None, :], (K, E))
    )
    return [
        {
            "outputs_in": outputs_t[b],
            "feats_in": feats_sh[b],
            "wT_in": wT,
            "bias_in": bias,
        }
        for b in range(B)
    ]


def kernel(outputs, feats, w_proj, b_proj, _trace=False, _trace_kwargs=None,
           _build_kwargs=None):
    key = tuple(sorted((_build_kwargs or {}).items()))
    if key not in _CACHE:
        _CACHE[key] = build_module(**(_build_kwargs or {}))
    nc = _CACHE[key]
    in_maps = make_in_maps(outputs, feats, w_proj, b_proj)
    res = run_bass_kernel_spmd(
        nc,
        in_maps,
        core_ids=list(range(N_CORES)),
        trace=_trace,
        **(_trace_kwargs or {}),
    )
    # out is [K, E] per sample; full output is [B, E, K]
    out = np.stack([np.asarray(r["out"]).T for r in res.results])
    if _trace:
        _CACHE["last_results"] = res
    return out


# revision 4
# speedup vs baseline: 1.3001x; 1.3001x over previous
"""Trainium2 Bass kernel for nn_Encoder segment-reduce.

Reference computation (per sample b):
    cls = onehot(argmax_k outputs[b])            # [K, HW]
    sizes = cls.sum(HW) + 0.01                   # [K]
    feat_set = feats[b] @ cls.T / sizes          # [F, K]
    out[b] = w_proj @ feat_set + bias            # [E, K]

Kernel strategy (pure data parallel: 1 sample per NeuronCore, 8 cores).

v4: mixed int8/bf16 feats, three-engine expansion, fgrp-major stream.

The kernel is jointly limited by (a) the feats HBM stream, (b) the PE pass
over feats (one moving column per f-column per 128-pixel chunk = 65536 PE
cycles ~ 27us, the dataflow floor), and (c) the on-chip int8->bf16
expansion rate.  bf16 feats alone make DMA the bottleneck (17MB ~ 50us);
int8 alone makes the cast engines the bottleneck (DVE ~407ns + ACT ~712ns
+ GpSimd ~1us per [128,512] chunk < the PE's 216ns/chunk appetite).  So
feats ship 28/32 chunks as int8 (host-quantized, scale 127/4.5) and 4/32
as bf16 pre-scaled by the same 127/4.5 (so 1/s folds into w_proj once);
the bf16 chunks sit at the end of each f-group, giving the cast engines a
catch-up window each quarter.  Final rel err ~8e-3.

outputs stay f32: a bf16 argmax flips ~141/32K pixels at class-assignment
ties, and one flipped pixel shifts a whole class mean - 0.13 rel err.

Loop order is fgrp-major (f-groups of 512 outer, hw chunks inner) so each
f-group's [21, 512] segment-reduce PSUM tile completes after its quarter of
the stream; its PSUM copy, PE transpose back to f-major, and projection
matmuls are interleaved into the FIRST HALF of the next quarter's stream.
Emission order per engine is chosen so no engine's queue ever waits on a
result produced later than ~1us after its queue position (a queued wait on
a far-future PE result stalls that engine's later casts, starves the PE,
and trips the HAM death spiral below).

Tail algebra: the per-class reciprocal commutes with the f-contraction, so
the projection accumulates raw sums into out^T [21, 256]; one recip
multiply plus one bias add (bias host-prebroadcast to [21, 256]) finish in
two DVE ops, and the store is a contiguous 1KB-per-partition DMA (host
transposes).

The onehot is 4 DVE instructions total: tensor_reduce over [P, t, 21] and
a broadcast is_equal via tensor_tensor, in two pieces so the first 8
chunks' onehot is ready early.

DMA: feats ride the sync HWDGE queue as 0.4-0.9MB sub-blocks (3.5-7KB
contiguous per partition); outputs/wT/bias/the out store ride the scalar
HWDGE queue so they never delay the feats stream.

HAM: the PE clock ramps 1.2->2.4GHz only under sustained load, and a
multi-us PE idle gap mid-kernel triggers a ~10us half-clock window that
slows EVERY engine (casts included) and spirals.  A warmup matmul burst
bridges the initial DMA window, and the schedule keeps PE duty near 100%
once streaming starts.
"""

import numpy as np

import concourse.bacc as bacc
import concourse.bass as bass
import concourse.mybir as mybir
import concourse.tile as tile
from concourse.bass import ds, ts
from concourse.bass_utils import run_bass_kernel_spmd
from concourse.masks import make_identity

# Problem shapes (hardcoded per contract)
B = 8
K = 21
H = 64
W = 64
HW = H * W            # 4096
F = 2048
E = 256
P = 128
FC = F // P           # 16 f-chunks of 128
FG = 4                # f-groups of 512 (psum accumulate tiles)
FGW = F // FG         # 512
N_T = HW // P         # 32 hw chunks
N_I8 = 28             # int8 chunks per fgrp (rest arrive bf16)
N_CORES = 8

F32 = mybir.dt.float32
BF16 = mybir.dt.bfloat16
I8 = mybir.dt.int8

QCLIP = 4.5
QSCALE = 127.0 / QCLIP

# cast-engine assignment for the 28 int8 chunks of each fgrp
# D=DVE (~407ns) x14, A=ACT (~712ns) x8, G=GpSimd (~1us) x6
CAST_PATTERN = "DGDADGADGDADAD" * 2


def build_module(warmup=75, cast_pattern=CAST_PATTERN):
    nc = bacc.Bacc("TRN2", target_bir_lowering=False, debug=False)

    # outputs host-transposed to [p, t, k] (pixel-major).
    outputs_d = nc.dram_tensor("outputs_in", [P, N_T, K], F32, kind="ExternalInput")
    # feats: int8 chunks t<28 and bf16 (pre-scaled by QSCALE) chunks 28..31,
    # both fgrp-major: [p, fgrp, t, fj]
    feats_i8_d = nc.dram_tensor("feats_i8", [P, FG, N_I8, FGW], I8,
                                kind="ExternalInput")
    feats_bf_d = nc.dram_tensor("feats_bf", [P, FG, N_T - N_I8, FGW], BF16,
                                kind="ExternalInput")
    # (w_proj / s).T rearranged [p, fc, e]
    wT_d = nc.dram_tensor("wT_in", [P, FC, E], BF16, kind="ExternalInput")
    # bias pre-broadcast to [k, e] on host
    bias_d = nc.dram_tensor("bias_in", [K, E], F32, kind="ExternalInput")
    # out^T = [k, e]; host transposes back
    out_d = nc.dram_tensor("out", [K, E], F32, kind="ExternalOutput")

    with tile.TileContext(nc) as tc:
        with (
            tc.tile_pool(name="consts", bufs=1) as consts,
            tc.tile_pool(name="fbf", bufs=1) as fbf,
            tc.tile_pool(name="small", bufs=4) as small,
            tc.tile_pool(name="ps_fs", bufs=1, space="PSUM") as ps_fs,
            tc.tile_pool(name="ps_out", bufs=1, space="PSUM") as ps_out,
            tc.tile_pool(name="ps_trp", bufs=1, space="PSUM") as ps_trp,
            tc.tile_pool(name="ps_misc", bufs=1, space="PSUM") as ps_misc,
        ):
            # ---- DMAs ------------------------------------------------
            # scalar HWDGE queue: everything except the feats stream.
            outputs_sb = consts.tile([P, N_T, K], F32)
            nc.scalar.dma_start(out=outputs_sb, in_=outputs_d.ap())
            bias_sb = consts.tile([K, E], F32)
            nc.scalar.dma_start(out=bias_sb, in_=bias_d.ap())
            wT_sb = consts.tile([P, FC, E], BF16)
            nc.scalar.dma_start(out=wT_sb, in_=wT_d.ap())

            # sync HWDGE queue: the feats stream.  fgrp 0's int8 part in
            # finer sub-blocks so the first casts can start sooner.
            feats_i8_sb = consts.tile([P, FG, N_I8, FGW], I8)
            feats_bf_sb = consts.tile([P, FG, N_T - N_I8, FGW], BF16)
            i8_r = feats_i8_d.ap()
            bf_r = feats_bf_d.ap()
            i8_blocks = {0: [(0, 7), (7, 14), (14, 21), (21, 28)],
                         1: [(0, 14), (14, 28)],
                         2: [(0, 14), (14, 28)],
                         3: [(0, 14), (14, 28)]}
            for g in range(FG):
                for (t0, t1) in i8_blocks[g]:
                    nc.sync.dma_start(
                        out=feats_i8_sb[:, g, ds(t0, t1 - t0)],
                        in_=i8_r[:, g, ds(t0, t1 - t0)],
                    )
                nc.sync.dma_start(
                    out=feats_bf_sb[:, g], in_=bf_r[:, g],
                )

            # ---- PE warm-up + constants ------------------------------
            warm_w = consts.tile([P, 64], BF16)
            nc.vector.memset(warm_w, 0.0)
            warm_ps = ps_misc.tile([P, 64], F32, tag="warm")
            for _ in range(warmup):
                nc.tensor.matmul(warm_ps[0:64, :], lhsT=warm_w, rhs=warm_w)

            # Preload the ACT engine's Copy activation table so the first
            # real cast doesn't eat the ~1.3us table load mid-stream.
            act_warm = small.tile([1, 1], BF16, tag="actw")
            nc.scalar.activation(out=act_warm, in_=warm_w[0:1, 0:1],
                                 func=mybir.ActivationFunctionType.Copy)

            ident = consts.tile([P, P], F32)
            make_identity(nc, ident)
            ident_b = consts.tile([K, K], BF16)
            nc.vector.tensor_copy(ident_b, ident[:K, :K])
            ones_b = consts.tile([P, 2], BF16)
            nc.vector.memset(ones_b, 1.0)

            # ---- onehot (DVE, 4 instructions in 2 pieces) ------------
            oh_all = consts.tile([P, N_T, K], BF16)
            rowmax = consts.tile([P, N_T, 1], F32)

            def emit_onehot(t0, t1):
                n = t1 - t0
                nc.vector.tensor_reduce(
                    rowmax[:, ds(t0, n)], outputs_sb[:, ds(t0, n)],
                    mybir.AxisListType.X, mybir.AluOpType.max,
                )
                nc.vector.tensor_tensor(
                    oh_all[:, ds(t0, n)], outputs_sb[:, ds(t0, n)],
                    rowmax[:, ds(t0, n)].to_broadcast((P, n, K)),
                    mybir.AluOpType.is_equal,
                )

            # ---- stream tiles ----------------------------------------
            fg_bf = [
                fbf.tile([P, N_I8, FGW], BF16, name=f"fgbf{i}", tag=f"fgbf{i}")
                for i in range(2)
            ]
            fs_ps = [
                ps_fs.tile([K, FGW], F32, name=f"fs{i}", tag=f"fs{i}")
                for i in range(2)
            ]
            fs_sc = consts.tile([K, F], BF16)
            fsT_sb = consts.tile([P, FC, K], BF16)
            sz_ps = ps_misc.tile([K, 2], F32, tag="sz")
            outT_ps = ps_out.tile([K, E], F32)

            def emit_cast(g, t):
                eng = cast_pattern[t]
                bf = fg_bf[g % 2]
                if eng == "D":
                    nc.vector.tensor_copy(bf[:, t, :], feats_i8_sb[:, g, t, :])
                elif eng == "G":
                    nc.gpsimd.tensor_copy(bf[:, t, :], feats_i8_sb[:, g, t, :])
                else:
                    nc.scalar.activation(
                        out=bf[:, t, :], in_=feats_i8_sb[:, g, t, :],
                        func=mybir.ActivationFunctionType.Copy,
                    )

            def emit_stream(g, t0, t1):
                bf = fg_bf[g % 2]
                for t in range(t0, t1):
                    rhs = (bf[:, t, :] if t < N_I8
                           else feats_bf_sb[:, g, t - N_I8, :])
                    nc.tensor.matmul(
                        fs_ps[g % 2], lhsT=oh_all[:, t, :], rhs=rhs,
                        start=(t == 0), stop=(t == N_T - 1),
                    )

            def emit_fs_copy(g):
                nc.vector.tensor_copy(fs_sc[:, ds(g * FGW, FGW)], fs_ps[g % 2])

            def emit_transposes(g):
                for j in range(4):
                    fc = g * 4 + j
                    trp = ps_trp.tile([P, K], BF16, name=f"trp{fc}",
                                      tag=f"trp{'AB'[fc % 2]}")
                    nc.tensor.transpose(trp, fs_sc[:, ts(fc, P)], ident_b)
                    nc.vector.tensor_copy(fsT_sb[:, fc, :], trp)

            def emit_projs(g):
                for j in range(4):
                    fc = g * 4 + j
                    nc.tensor.matmul(
                        outT_ps, lhsT=fsT_sb[:, fc, :], rhs=wT_sb[:, fc, :],
                        start=(fc == 0), stop=(fc == FC - 1),
                    )

            # ---- main schedule ---------------------------------------
            # fgrp 0: onehot piece A, early casts, stream starts; the 32
            # sizes matmuls fill the PE while casts get ahead.
            emit_onehot(0, 8)
            for t in range(8):
                emit_cast(0, t)
            emit_onehot(8, N_T)
            emit_stream(0, 0, 8)
            for t in range(N_T):
                nc.tensor.matmul(
                    sz_ps, lhsT=oh_all[:, t, :], rhs=ones_b,
                    start=(t == 0), stop=(t == N_T - 1),
                )
            for t in range(8, N_I8):
                emit_cast(0, t)
            emit_stream(0, 8, N_T)

            # fgrps 1..3: previous fgrp's copy/transpose/proj interleave
            # into the first half of this fgrp's stream.
            for g in range(1, FG):
                for t in range(8):
                    emit_cast(g, t)
                emit_fs_copy(g - 1)
                emit_stream(g, 0, 8)
                emit_transposes(g - 1)
                for t in range(8, 16):
                    emit_cast(g, t)
                emit_stream(g, 8, 16)
                emit_projs(g - 1)
                for t in range(16, N_I8):
                    emit_cast(g, t)
                emit_stream(g, 16, N_T)

            emit_fs_copy(FG - 1)
            emit_transposes(FG - 1)
            emit_projs(FG - 1)

            # ---- tail ------------------------------------------------
            sizes_sb = small.tile([K, 1], F32, tag="sizes")
            nc.vector.tensor_scalar_add(sizes_sb, sz_ps[:, 0:1], 0.01)
            recip = small.tile([K, 1], F32, tag="recip")
            nc.vector.reciprocal(recip, sizes_sb)
            out_sb = consts.tile([K, E], F32)
            nc.vector.tensor_scalar_mul(out_sb, outT_ps, recip)
            nc.vector.tensor_add(out_sb, out_sb, bias_sb)
            nc.scalar.dma_start(out=out_d.ap(), in_=out_sb)

    nc.compile()
    return nc


_CACHE = {}


def make_in_maps(outputs, feats, w_proj, b_proj):
    import ml_dtypes

    outputs = np.asarray(outputs, dtype=np.float32)
    # [B, K, H, W] -> per sample [p, t, k] (pixel-major: hw = t*128 + p)
    outputs_t = np.ascontiguousarray(
        outputs.reshape(B, K, N_T, P).transpose(0, 3, 2, 1)
    )
    feats = np.asarray(feats, dtype=np.float32)
    # [B, F, H, W] -> [b, g, fj, t, p]; chunks t<28 int8, t>=28 bf16*QSCALE
    f5 = feats.reshape(B, FG, FGW, N_T, P)
    q = np.clip(np.round(f5[:, :, :, :N_I8] * QSCALE), -127, 127).astype(np.int8)
    feats_i8 = np.ascontiguousarray(q.transpose(0, 4, 1, 3, 2))
    fbf = (f5[:, :, :, N_I8:] * QSCALE).astype(ml_dtypes.bfloat16)
    feats_bf = np.ascontiguousarray(fbf.transpose(0, 4, 1, 3, 2))
    wT = np.ascontiguousarray(
        (np.asarray(w_proj, dtype=np.float32).T / QSCALE)
        .reshape(FC, P, E).transpose(1, 0, 2)
        .astype(ml_dtypes.bfloat16)
    )
    bias = np.ascontiguousarray(
        np.broadcast_to(np.asarray(b_proj, dtype=np.float32)[None, :], (K, E))
    )
    return [
        {
            "outputs_in": outputs_t[b],
            "feats_i8": feats_i8[b],
            "feats_bf": feats_bf[b],
            "wT_in": wT,
            "bias_in": bias,
        }
        for b in range(B)
    ]


def kernel(outputs, feats, w_proj, b_proj, _trace=False, _trace_kwargs=None,
           _build_kwargs=None):
    key = tuple(sorted((_build_kwargs or {}).items()))
    if key not in _CACHE:
        _CACHE[key] = build_module(**(_build_kwargs or {}))
    nc = _CACHE[key]
    in_maps = make_in_maps(outputs, feats, w_proj, b_proj)
    res = run_bass_kernel_spmd(
        nc,
        in_maps,
        core_ids=list(range(N_CORES)),
        trace=_trace,
        **(_trace_kwargs or {}),
    )
    # out is [K, E] per sample; full output is [B, E, K]
    out = np.stack([np.asarray(r["out"]).T for r in res.results])
    if _trace:
        _CACHE["last_results"] = res
    return out


# revision 13
# speedup vs baseline: 1.5375x; 1.1826x over previous
"""Trainium2 Bass kernel for nn_Encoder segment-reduce.

Reference computation (per sample b):
    cls = onehot(argmax_k outputs[b])            # [K, HW]
    sizes = cls.sum(HW) + 0.01                   # [K]
    feat_set = feats[b] @ cls.T / sizes          # [F, K]
    out[b] = w_proj @ feat_set + bias            # [E, K]

Kernel strategy (pure data parallel: 1 sample per NeuronCore, 8 cores).

v4: mixed int8/bf16 feats, three-engine expansion, fgrp-major stream.

The kernel is jointly limited by (a) the feats HBM stream, (b) the PE pass
over feats (one moving column per f-column per 128-pixel chunk = 65536 PE
cycles ~ 27us, the dataflow floor), and (c) the on-chip int8->bf16
expansion rate.  bf16 feats alone make DMA the bottleneck (17MB ~ 50us);
int8 alone makes the cast engines the bottleneck (DVE ~407ns + ACT ~712ns
+ GpSimd ~1us per [128,512] chunk < the PE's 216ns/chunk appetite).  So
feats ship 28/32 chunks as int8 (host-quantized, scale 127/4.5) and 4/32
as bf16 pre-scaled by the same 127/4.5 (so 1/s folds into w_proj once);
the bf16 chunks sit at the end of each f-group, giving the cast engines a
catch-up window each quarter.  Final rel err ~8e-3.

outputs stay f32: a bf16 argmax flips ~141/32K pixels at class-assignment
ties, and one flipped pixel shifts a whole class mean - 0.13 rel err.

Loop order is fgrp-major (f-groups of 512 outer, hw chunks inner) so each
f-group's [21, 512] segment-reduce PSUM tile completes after its quarter of
the stream; its PSUM copy, PE transpose back to f-major, and projection
matmuls are interleaved into the FIRST HALF of the next quarter's stream.
Emission order per engine is chosen so no engine's queue ever waits on a
result produced later than ~1us after its queue position (a queued wait on
a far-future PE result stalls that engine's later casts, starves the PE,
and trips the HAM death spiral below).

Tail algebra: the per-class reciprocal commutes with the f-contraction, so
the projection accumulates raw sums into out^T [21, 256]; one recip
multiply plus one bias add (bias host-prebroadcast to [21, 256]) finish in
two DVE ops, and the store is a contiguous 1KB-per-partition DMA (host
transposes).

The onehot is 4 DVE instructions total: tensor_reduce over [P, t, 21] and
a broadcast is_equal via tensor_tensor, in two pieces so the first 8
chunks' onehot is ready early.

DMA: feats ride the sync HWDGE queue as 0.4-0.9MB sub-blocks (3.5-7KB
contiguous per partition); outputs/wT/bias/the out store ride the scalar
HWDGE queue so they never delay the feats stream.

HAM: the PE clock ramps 1.2->2.4GHz only under sustained load, and a
multi-us PE idle gap mid-kernel triggers a ~10us half-clock window that
slows EVERY engine (casts included) and spirals.  A warmup matmul burst
bridges the initial DMA window, and the schedule keeps PE duty near 100%
once streaming starts.
"""

import numpy as np

import concourse.bacc as bacc
import concourse.bass as bass
import concourse.mybir as mybir
import concourse.tile as tile
from concourse.bass import ds, ts
from concourse.bass_utils import run_bass_kernel_spmd
from concourse.masks import make_identity

# Problem shapes (hardcoded per contract)
B = 8
K = 21
H = 64
W = 64
HW = H * W            # 4096
F = 2048
E = 256
P = 128
FC = F // P           # 16 f-chunks of 128
FG = 4                # f-groups of 512 (psum accumulate tiles)
FGW = F // FG         # 512
N_T = HW // P         # 32 hw chunks
N_CORES = 8

F32 = mybir.dt.float32
BF16 = mybir.dt.bfloat16
I8 = mybir.dt.int8

QCLIP = 4.5
QSCALE = 127.0 / QCLIP

# int8 chunks per fgrp (rest arrive bf16-direct at the fgrp's end):
# fgrp 0 gets extra bf16-direct chunks so the PE never outruns the cast
# engines before the pipeline has built a lead.
N_I8_G = [24, 27, 27, 27]


def _mk_pattern(n, na, g_pos):
    """Cast-engine pattern: D=DVE (~407ns), A=ACT (~712ns), G=GpSimd
    (~2us! - only a few, at low-urgency positions); A spread evenly."""
    s = ["D"] * n
    for p in g_pos:
        s[p] = "G"
    rest = [i for i in range(n) if s[i] == "D"]
    for j in range(na):
        s[rest[(2 * j + 1) * len(rest) // (2 * na)]] = "A"
    return "".join(s)


CAST_PATTERNS = [
    _mk_pattern(24, 9, (17, 21)),
    _mk_pattern(27, 9, (8, 17, 25)),
    _mk_pattern(27, 9, (8, 17, 25)),
    _mk_pattern(27, 9, (8, 17, 25)),
]


def build_module(warmup=75, fillers=8):
    nc = bacc.Bacc("TRN2", target_bir_lowering=False, debug=False)

    # outputs host-transposed to [p, t, k] (pixel-major).
    outputs_d = nc.dram_tensor("outputs_in", [P, N_T, K], F32, kind="ExternalInput")
    # feats per fgrp: int8 chunks t < N_I8_G[g], bf16 (pre-scaled by
    # QSCALE) for the rest; [p, t, fj]
    feats_i8_d = [
        nc.dram_tensor(f"feats_i8_{g}", [P, N_I8_G[g], FGW], I8,
                       kind="ExternalInput")
        for g in range(FG)
    ]
    feats_bf_d = [
        nc.dram_tensor(f"feats_bf_{g}", [P, N_T - N_I8_G[g], FGW], BF16,
                       kind="ExternalInput")
        for g in range(FG)
    ]
    # (w_proj / s).T rearranged [p, fc, e]
    wT_d = nc.dram_tensor("wT_in", [P, FC, E], BF16, kind="ExternalInput")
    # bias pre-broadcast to [k, e] on host
    bias_d = nc.dram_tensor("bias_in", [K, E], F32, kind="ExternalInput")
    # out^T = [k, e]; host transposes back
    out_d = nc.dram_tensor("out", [K, E], F32, kind="ExternalOutput")

    with tile.TileContext(nc) as tc:
        with (
            tc.tile_pool(name="consts", bufs=1) as consts,
            tc.tile_pool(name="fbf", bufs=1) as fbf,
            tc.tile_pool(name="small", bufs=4) as small,
            tc.tile_pool(name="ps_fs", bufs=1, space="PSUM") as ps_fs,
            tc.tile_pool(name="ps_out", bufs=1, space="PSUM") as ps_out,
            tc.tile_pool(name="ps_trp", bufs=1, space="PSUM") as ps_trp,
            tc.tile_pool(name="ps_misc", bufs=1, space="PSUM") as ps_misc,
        ):
            # ---- DMAs ------------------------------------------------
            # scalar HWDGE queue: everything except the feats stream.
            outputs_sb = consts.tile([P, N_T, K], F32)
            nc.scalar.dma_start(out=outputs_sb, in_=outputs_d.ap())
            bias_sb = consts.tile([K, E], F32)
            nc.scalar.dma_start(out=bias_sb, in_=bias_d.ap())
            wT_sb = consts.tile([P, FC, E], BF16)
            nc.scalar.dma_start(out=wT_sb, in_=wT_d.ap())

            # sync HWDGE queue: the feats stream.  fgrp 0's int8 part in
            # finer sub-blocks so the first casts can start sooner.
            feats_i8_sb = [
                consts.tile([P, N_I8_G[g], FGW], I8, name=f"fi8_{g}")
                for g in range(FG)
            ]
            feats_bf_sb = [
                consts.tile([P, N_T - N_I8_G[g], FGW], BF16, name=f"fbfd_{g}")
                for g in range(FG)
            ]
            i8_blocks = {0: [(0, 6), (6, 12), (12, 18), (18, 24)],
                         1: [(0, 14), (14, 27)],
                         2: [(0, 14), (14, 27)],
                         3: [(0, 14), (14, 27)]}
            for g in range(FG):
                for (t0, t1) in i8_blocks[g]:
                    nc.sync.dma_start(
                        out=feats_i8_sb[g][:, ds(t0, t1 - t0)],
                        in_=feats_i8_d[g].ap()[:, ds(t0, t1 - t0)],
                    )
                nc.sync.dma_start(
                    out=feats_bf_sb[g], in_=feats_bf_d[g].ap(),
                )

            # ---- PE warm-up + constants ------------------------------
            warm_w = consts.tile([P, FGW], BF16)
            nc.vector.memset(warm_w, 0.0)
            warm_ps = ps_misc.tile([P, 64], F32, tag="warm")
            # N=512 filler matmuls hold PE duty at 100% while the cast
            # pipeline builds its lead during fgrp 0 (HAM insurance).
            warm_ps512 = ps_misc.tile([64, FGW], F32, tag="warm512")

            def emit_filler():
                nc.tensor.matmul(warm_ps512, lhsT=warm_w[:, 0:64], rhs=warm_w)

            for _ in range(warmup):
                nc.tensor.matmul(warm_ps[0:64, :], lhsT=warm_w[:, 0:64],
                                 rhs=warm_w[:, 0:64])

            # Preload the ACT engine's Copy activation table so the first
            # real cast doesn't eat the ~1.3us table load mid-stream.
            act_warm = small.tile([1, 1], BF16, tag="actw")
            nc.scalar.activation(out=act_warm, in_=warm_w[0:1, 0:1],
                                 func=mybir.ActivationFunctionType.Copy)

            ident = consts.tile([P, P], F32)
            make_identity(nc, ident)
            ident_b = consts.tile([K, K], BF16)
            nc.vector.tensor_copy(ident_b, ident[:K, :K])
            ones_b = consts.tile([P, 2], BF16)
            nc.vector.memset(ones_b, 1.0)

            # ---- onehot (DVE, 4 instructions in 2 pieces) ------------
            oh_all = consts.tile([P, N_T, K], BF16)
            rowmax = consts.tile([P, N_T, 1], F32)

            def emit_onehot(t0, t1):
                n = t1 - t0
                nc.vector.tensor_reduce(
                    rowmax[:, ds(t0, n)], outputs_sb[:, ds(t0, n)],
                    mybir.AxisListType.X, mybir.AluOpType.max,
                )
                nc.vector.tensor_tensor(
                    oh_all[:, ds(t0, n)], outputs_sb[:, ds(t0, n)],
                    rowmax[:, ds(t0, n)].to_broadcast((P, n, K)),
                    mybir.AluOpType.is_equal,
                )

            # ---- stream tiles ----------------------------------------
            fg_bf = [
                fbf.tile([P, max(N_I8_G), FGW], BF16, name=f"fgbf{i}",
                         tag=f"fgbf{i}")
                for i in range(2)
            ]
            fs_ps = [
                ps_fs.tile([K, FGW], F32, name=f"fs{i}", tag=f"fs{i}")
                for i in range(2)
            ]
            fs_sc = consts.tile([K, F], BF16)
            fsT_sb = consts.tile([P, FC, K], BF16)
            sz_ps = ps_misc.tile([K, 2], F32, tag="sz")
            outT_ps = ps_out.tile([K, E], F32)

            def emit_cast(g, t):
                eng = CAST_PATTERNS[g][t]
                bf = fg_bf[g % 2]
                if eng == "D":
                    nc.vector.tensor_copy(bf[:, t, :], feats_i8_sb[g][:, t, :])
                elif eng == "G":
                    nc.gpsimd.tensor_copy(bf[:, t, :], feats_i8_sb[g][:, t, :])
                else:
                    nc.scalar.activation(
                        out=bf[:, t, :], in_=feats_i8_sb[g][:, t, :],
                        func=mybir.ActivationFunctionType.Copy,
                    )

            def emit_stream(g, t0, t1, filler_until=-1):
                bf = fg_bf[g % 2]
                n_i8 = N_I8_G[g]
                for t in range(t0, t1):
                    rhs = (bf[:, t, :] if t < n_i8
                           else feats_bf_sb[g][:, t - n_i8, :])
                    nc.tensor.matmul(
                        fs_ps[g % 2], lhsT=oh_all[:, t, :], rhs=rhs,
                        start=(t == 0), stop=(t == N_T - 1),
                    )
                    if t < filler_until and t % 2 == 1:
                        emit_filler()

            def emit_fs_copy(g):
                nc.vector.tensor_copy(fs_sc[:, ds(g * FGW, FGW)], fs_ps[g % 2])

            def emit_transposes(g):
                for j in range(4):
                    fc = g * 4 + j
                    trp = ps_trp.tile([P, K], BF16, name=f"trp{fc}",
                                      tag=f"trp{'AB'[fc % 2]}")
                    nc.tensor.transpose(trp, fs_sc[:, ts(fc, P)], ident_b)
                    nc.vector.tensor_copy(fsT_sb[:, fc, :], trp)

            def emit_projs(g):
                for j in range(4):
                    fc = g * 4 + j
                    nc.tensor.matmul(
                        outT_ps, lhsT=fsT_sb[:, fc, :], rhs=wT_sb[:, fc, :],
                        start=(fc == 0), stop=(fc == FC - 1),
                    )

            # ---- main schedule ---------------------------------------
            # fgrp 0: onehot piece A, early casts, stream starts; the 32
            # sizes matmuls fill the PE while casts get ahead.
            emit_onehot(0, 8)
            for t in range(8):
                emit_cast(0, t)
            emit_onehot(8, N_T)
            emit_stream(0, 0, 8, filler_until=2 * fillers)
            for t in range(N_T):
                nc.tensor.matmul(
                    sz_ps, lhsT=oh_all[:, t, :], rhs=ones_b,
                    start=(t == 0), stop=(t == N_T - 1),
                )
            for t in range(8, N_I8_G[0]):
                emit_cast(0, t)
            emit_stream(0, 8, N_T, filler_until=2 * fillers)

            # fgrps 1..3: previous fgrp's copy/transpose/proj interleave
            # into the first half of this fgrp's stream.
            for g in range(1, FG):
                for t in range(8):
                    emit_cast(g, t)
                emit_fs_copy(g - 1)
                emit_stream(g, 0, 8)
                emit_transposes(g - 1)
                for t in range(8, 16):
                    emit_cast(g, t)
                emit_stream(g, 8, 16)
                emit_projs(g - 1)
                for t in range(16, N_I8_G[g]):
                    emit_cast(g, t)
                emit_stream(g, 16, N_T)

            emit_fs_copy(FG - 1)
            emit_transposes(FG - 1)
            emit_projs(FG - 1)

            # ---- tail ------------------------------------------------
            sizes_sb = small.tile([K, 1], F32, tag="sizes")
            nc.vector.tensor_scalar_add(sizes_sb, sz_ps[:, 0:1], 0.01)
            recip = small.tile([K, 1], F32, tag="recip")
            nc.vector.reciprocal(recip, sizes_sb)
            out_sb = consts.tile([K, E], F32)
            nc.vector.tensor_scalar_mul(out_sb, outT_ps, recip)
            nc.vector.tensor_add(out_sb, out_sb, bias_sb)
            nc.scalar.dma_start(out=out_d.ap(), in_=out_sb)

    nc.compile()
    return nc


_CACHE = {}


def make_in_maps(outputs, feats, w_proj, b_proj):
    import ml_dtypes

    outputs = np.asarray(outputs, dtype=np.float32)
    # [B, K, H, W] -> per sample [p, t, k] (pixel-major: hw = t*128 + p)
    outputs_t = np.ascontiguousarray(
        outputs.reshape(B, K, N_T, P).transpose(0, 3, 2, 1)
    )
    feats = np.asarray(feats, dtype=np.float32)
    # [B, F, H, W] -> [b, g, fj, t, p]; per fgrp chunks t < N_I8_G[g] int8,
    # the rest bf16*QSCALE
    f5 = feats.reshape(B, FG, FGW, N_T, P)
    feats_i8 = {}
    feats_bf = {}
    for g in range(FG):
        n = N_I8_G[g]
        q = np.clip(np.round(f5[:, g, :, :n] * QSCALE), -127, 127).astype(np.int8)
        feats_i8[g] = np.ascontiguousarray(q.transpose(0, 3, 2, 1))
        fbf = (f5[:, g, :, n:] * QSCALE).astype(ml_dtypes.bfloat16)
        feats_bf[g] = np.ascontiguousarray(fbf.transpose(0, 3, 2, 1))
    wT = np.ascontiguousarray(
        (np.asarray(w_proj, dtype=np.float32).T / QSCALE)
        .reshape(FC, P, E).transpose(1, 0, 2)
        .astype(ml_dtypes.bfloat16)
    )
    bias = np.ascontiguousarray(
        np.broadcast_to(np.asarray(b_proj, dtype=np.float32)[None, :], (K, E))
    )
    maps = []
    for b in range(B):
        m = {"outputs_in": outputs_t[b], "wT_in": wT, "bias_in": bias}
        for g in range(FG):
            m[f"feats_i8_{g}"] = feats_i8[g][b]
            m[f"feats_bf_{g}"] = feats_bf[g][b]
        maps.append(m)
    return maps


def kernel(outputs, feats, w_proj, b_proj, _trace=False, _trace_kwargs=None,
           _build_kwargs=None):
    key = tuple(sorted((_build_kwargs or {}).items()))
    if key not in _CACHE:
        _CACHE[key] = build_module(**(_build_kwargs or {}))
    nc = _CACHE[key]
    in_maps = make_in_maps(outputs, feats, w_proj, b_proj)
    res = run_bass_kernel_spmd(
        nc,
        in_maps,
        core_ids=list(range(N_CORES)),
        trace=_trace,
        **(_trace_kwargs or {}),
    )
    # out is [K, E] per sample; full output is [B, E, K]
    out = np.stack([np.asarray(r["out"]).T for r in res.results])
    if _trace:
        _CACHE["last_results"] = res
    return out
